# revision 8
# baseline (speedup 1.0000x reference)
"""AxisNet Trainium2 kernel.

Sharding: batch-parallel, one batch element per NeuronCore (B=8 = n_cores).
Per core: N=4096 points x K=32 neighbors = 131072 (point,neighbor) pairs,
split into 4 chunks of 32768 pairs living on partition groups [32j:32j+32].

Conv stack (9 shared-MLP layers, training-mode BN): channels-on-partitions,
4-way-concurrent diagonal tile_position matmuls, fp16 activations, BN affine
folded so only relu(z + C) with per-channel C touches the data path (the BN
scale s is folded into the next layer's weights on device).  BN stats are
computed exactly with DVE bn_stats/bn_aggr on the fp32 PSUM conv outputs and
all-reduced across the 8 cores.  Max-pool over K is done on raw conv-9 output
(valid because s>0), then 3 FC layers + BN, final axis Gram-Schmidt on host.
"""

import os
import sys
import numpy as np

B, N, K = 8, 4096, 32
NCORES = 8
PAIRS = N * K            # 131072 per core
CHUNK = PAIRS // 4       # 32768
PTS_PER_CHUNK = N // 4   # 1024
BN_EPS = 1e-5
MACRO = 2048             # pass-A macro tile (4 psum banks)
NMACRO = CHUNK // MACRO  # 16
NSLOT = CHUNK // 512     # 64 bn_stats slots per conv layer

# (c_main, o, has_xyz); L0 reads gx directly
CONV = [
    (3, 6, False), (6, 16, False), (16, 16, False),
    (16, 16, True), (16, 16, False), (16, 32, False),
    (32, 32, True), (32, 32, False), (32, 64, False),
]

_CACHE = {}


def _build():
    sys.path.insert(0, "/opt/trn_rl_repo")
    import concourse.bacc as bacc
    import concourse.mybir as mybir
    import concourse.tile as tile

    dt = mybir.dt
    AF = mybir.ActivationFunctionType
    OP = mybir.AluOpType

    nc = bacc.Bacc("TRN2", target_bir_lowering=False, debug=False,
                   enable_asserts=False, num_devices=NCORES)

    # ---------------- DRAM I/O ----------------
    gx_d = nc.dram_tensor("gx", [4, 3, CHUNK], dt.float32, kind="ExternalInput")
    wm_d = [nc.dram_tensor(f"wm{i}", [128, 64 if i == 8 else 32], dt.float32,
                           kind="ExternalInput") for i in range(9)]
    wx_d = {i: nc.dram_tensor(f"wx{i}", [128, 32], dt.float32,
                              kind="ExternalInput") for i in (3, 6)}
    wfc1_d = nc.dram_tensor("wfc1", [128, 32], dt.float32, kind="ExternalInput")
    wfc2_d = nc.dram_tensor("wfc2", [128, 32], dt.float32, kind="ExternalInput")
    wfc3_d = nc.dram_tensor("wfc3", [128, 8], dt.float32, kind="ExternalInput")
    m32_d = nc.dram_tensor("m32", [128, 32], dt.float32, kind="ExternalInput")
    m64_d = nc.dram_tensor("m64", [128, 64], dt.float32, kind="ExternalInput")
    mfc_d = nc.dram_tensor("mfc", [64, 32], dt.float32, kind="ExternalInput")
    bc32_d = nc.dram_tensor("bc32", [32, 128], dt.float32, kind="ExternalInput")
    bc64_d = nc.dram_tensor("bc64", [64, 128], dt.float32, kind="ExternalInput")
    bcfc_d = nc.dram_tensor("bcfc", [32, 64], dt.float32, kind="ExternalInput")
    gam_d = nc.dram_tensor("gam", [64, 11], dt.float32, kind="ExternalInput")
    big_d = nc.dram_tensor("big", [64, 11], dt.float32, kind="ExternalInput")
    bfc3_d = nc.dram_tensor("bfc3", [64, 1], dt.float32, kind="ExternalInput")
    out_d = nc.dram_tensor("xout", [6, 4096], dt.float32, kind="ExternalOutput")

    with tile.TileContext(nc) as tc:
        with tc.tile_pool(name="big", bufs=1) as bigp, \
             tc.tile_pool(name="consts", bufs=1) as cons, \
             tc.tile_pool(name="weff", bufs=2) as weffp, \
             tc.tile_pool(name="gxp", bufs=2) as gxp, \
             tc.tile_pool(name="stat", bufs=1) as statp, \
             tc.tile_pool(name="tiny", bufs=2) as tiny, \
             tc.tile_pool(name="dram", bufs=2, space="DRAM") as drp:

            # --------- resident tensors ---------
            X = bigp.tile([128, CHUNK], dt.float32, tag="X")

            wm = []
            for i in range(9):
                w = cons.tile([128, 64 if i == 8 else 32], dt.float32, tag=f"wm{i}", name=f"wm{i}")
                nc.sync.dma_start(w[:], wm_d[i].ap())
                wm.append(w)
            wx = {}
            for i in (3, 6):
                wx[i] = cons.tile([128, 32], dt.float32, tag=f"wx{i}", name=f"wx{i}")
                nc.sync.dma_start(wx[i][:], wx_d[i].ap())
            wfc1 = cons.tile([128, 32], dt.float32, tag="wfc1")
            wfc2 = cons.tile([128, 32], dt.float32, tag="wfc2")
            wfc3 = cons.tile([128, 8], dt.float32, tag="wfc3")
            nc.sync.dma_start(wfc1[:], wfc1_d.ap())
            nc.sync.dma_start(wfc2[:], wfc2_d.ap())
            nc.sync.dma_start(wfc3[:], wfc3_d.ap())
            m32 = cons.tile([128, 32], dt.float32, tag="m32")
            m64 = cons.tile([128, 64], dt.float32, tag="m64")
            mfc = cons.tile([64, 32], dt.float32, tag="mfc")
            bc32 = cons.tile([32, 128], dt.float32, tag="bc32")
            bc64 = cons.tile([64, 128], dt.float32, tag="bc64")
            bcfc = cons.tile([32, 64], dt.float32, tag="bcfc")
            gam = cons.tile([64, 11], dt.float32, tag="gam")
            big = cons.tile([64, 11], dt.float32, tag="big")
            bfc3 = cons.tile([64, 1], dt.float32, tag="bfc3")
            for t, d in ((m32, m32_d), (m64, m64_d), (mfc, mfc_d),
                         (bc32, bc32_d), (bc64, bc64_d), (bcfc, bcfc_d),
                         (gam, gam_d), (big, big_d), (bfc3, bfc3_d)):
                nc.sync.dma_start(t[:], d.ap())

            pooled = [bigp.tile([128, 1024], dt.float32, tag=f"pool{s}", name=f"pool{s}")
                      for s in range(2)]


            # ---------- stats tail: returns (C_bcast[PB,1]f32, s[m,1]f32) ----------
            def stats_tail(stats_ap, m, G, bn_idx, mask, bcmask, psp, pstag, PB):
                aggr = tiny.tile([128, 2], dt.float32, tag="aggr")
                nc.vector.bn_aggr(aggr[0:stats_ap.shape[0], :], stats_ap)
                # m2 = var + mean^2 (per partition)
                a2 = tiny.tile([128, 2], dt.float32, tag="a2")
                P = stats_ap.shape[0]
                nc.vector.tensor_tensor(a2[0:P, 0:1], aggr[0:P, 0:1],
                                        aggr[0:P, 0:1], OP.mult)
                nc.vector.tensor_tensor(a2[0:P, 1:2], a2[0:P, 0:1],
                                        aggr[0:P, 1:2], OP.add)
                nc.vector.tensor_copy(a2[0:P, 0:1], aggr[0:P, 0:1])
                # sum across groups via mask matmul (fp32)
                psr = psp.tile(psp_shape(pstag), dt.float32, tag=pstag, name="psr")
                nc.tensor.matmul(psr[0:m, 0:2], mask[0:P, 0:m], a2[0:P, 0:2],
                                 start=True, stop=True)
                red = tiny.tile([128, 2], dt.float32, tag="red")
                nc.scalar.activation(red[0:m, 0:2], psr[0:m, 0:2], AF.Copy)
                # cross-core all-reduce
                bin_ = drp.tile([64, 2], dt.float32, tag="ccin")
                bout = drp.tile([64, 2], dt.float32, tag="ccout")
                nc.sync.dma_start(bin_[0:m, :], red[0:m, 0:2])
                nc.gpsimd.collective_compute(
                    "AllReduce", OP.add,
                    replica_groups=[list(range(NCORES))],
                    ins=[bin_.opt()], outs=[bout.opt()])
                rsum = tiny.tile([64, 2], dt.float32, tag="rsum")
                nc.sync.dma_start(rsum[0:m, :], bout[0:m, :])
                inv = 1.0 / (G * NCORES)
                mu = tiny.tile([64, 1], dt.float32, tag="mu")
                m2g = tiny.tile([64, 1], dt.float32, tag="m2g")
                nc.vector.tensor_scalar(mu[0:m, :], rsum[0:m, 0:1], inv, None, OP.mult)
                nc.vector.tensor_scalar(m2g[0:m, :], rsum[0:m, 1:2], inv, None, OP.mult)
                var = tiny.tile([64, 1], dt.float32, tag="var")
                nc.vector.tensor_tensor(var[0:m, :], mu[0:m, :], mu[0:m, :], OP.mult)
                nc.vector.tensor_tensor(var[0:m, :], m2g[0:m, :], var[0:m, :],
                                        OP.subtract)
                nc.vector.tensor_scalar(var[0:m, :], var[0:m, :], BN_EPS, None, OP.add)
                # r = sqrt(var+eps) with one Newton step
                r = tiny.tile([64, 1], dt.float32, tag="r")
                nc.scalar.activation(r[0:m, :], var[0:m, :], AF.Sqrt)
                ir = tiny.tile([64, 1], dt.float32, tag="ir")
                nc.vector.reciprocal(ir[0:m, :], r[0:m, :])
                t1 = tiny.tile([64, 1], dt.float32, tag="t1")
                nc.vector.tensor_tensor(t1[0:m, :], var[0:m, :], ir[0:m, :], OP.mult)
                nc.vector.tensor_tensor(r[0:m, :], r[0:m, :], t1[0:m, :], OP.add)
                nc.vector.tensor_scalar(r[0:m, :], r[0:m, :], 0.5, None, OP.mult)
                # s = gamma / r ; C = (beta/gamma)*r - mu
                s = tiny.tile([64, 1], dt.float32, tag="s")
                nc.vector.reciprocal(ir[0:m, :], r[0:m, :])
                nc.vector.tensor_tensor(s[0:m, :], gam[0:m, bn_idx:bn_idx + 1],
                                        ir[0:m, :], OP.mult)
                Cv = tiny.tile([64, 1], dt.float32, tag="Cv")
                nc.vector.tensor_tensor(Cv[0:m, :], big[0:m, bn_idx:bn_idx + 1],
                                        r[0:m, :], OP.mult)
                nc.vector.tensor_tensor(Cv[0:m, :], Cv[0:m, :], mu[0:m, :],
                                        OP.subtract)
                # broadcast C to [PB,1] via mask matmul
                psb = psp.tile(psp_shape(pstag), dt.float32, tag=pstag, name="psb")
                nc.tensor.matmul(psb[0:PB, 0:1], bcmask[0:m, 0:PB], Cv[0:m, 0:1],
                                 start=True, stop=True)
                Cb = tiny.tile([128, 1], dt.float32, tag="Cb")
                nc.scalar.activation(Cb[0:PB, :], psb[0:PB, 0:1], AF.Copy)
                return Cb, s

            def bcast_s(s, m, bcmask, psp, pstag, PB, tag):
                psb = psp.tile(psp_shape(pstag), dt.float32, tag=pstag, name="psb")
                nc.tensor.matmul(psb[0:PB, 0:1], bcmask[0:m, 0:PB], s[0:m, 0:1],
                                 start=True, stop=True)
                sb = tiny.tile([128, 1], dt.float32, tag=tag, name=tag)
                nc.scalar.activation(sb[0:PB, :], psb[0:PB, 0:1], AF.Copy)
                return sb

            psp_shape = lambda tag: {"psA": [128, MACRO], "ps9": [128, 512],
                                     "psfc": [128, 512]}[tag]

            # ================= conv layers 0..7 =================
            with tc.tile_pool(name="psA", bufs=2, space="PSUM") as psp:
                sprev = None  # s broadcast [128,1] of previous layer
                for li in range(8):
                    c, o, hasx = CONV[li]
                    stats = statp.tile([128, NSLOT, 6], dt.float32, tag="cstats")
                    if li == 0:
                        weff = wm[0]
                    else:
                        weff = weffp.tile([128, 32], dt.float32, tag="weff")
                        nc.vector.tensor_scalar(weff[:], wm[li][:], sprev[:, 0:1],
                                                None, OP.mult)
                    needs_gx = (li == 0) or hasx
                    for t in range(NMACRO):
                        ps = psp.tile([128, MACRO], dt.float32, tag="psA")
                        c0 = t * MACRO
                        if needs_gx:
                            gxs = gxp.tile([128, MACRO], dt.float32, tag="gxs",
                                           name="gxs")
                            for j in range(4):
                                nc.sync.dma_start(
                                    gxs[32 * j:32 * j + 3, :],
                                    gx_d.ap()[j, :, c0:c0 + MACRO])
                        for b in range(4):
                            cb = c0 + 512 * b
                            lb = 512 * b
                            for j in range(4):
                                r0 = 32 * j
                                if li == 0:
                                    nc.tensor.matmul(
                                        ps[r0:r0 + 32, lb:lb + 512],
                                        weff[r0:r0 + c, 0:32],
                                        gxs[r0:r0 + c, lb:lb + 512],
                                        start=True, stop=True,
                                        tile_position=(r0, r0))
                                else:
                                    nc.tensor.matmul(
                                        ps[r0:r0 + 32, lb:lb + 512],
                                        weff[r0:r0 + c, 0:32],
                                        X[r0:r0 + c, cb:cb + 512],
                                        start=True, stop=not hasx,
                                        tile_position=(r0, r0))
                                    if hasx:
                                        nc.tensor.matmul(
                                            ps[r0:r0 + 32, lb:lb + 512],
                                            wx[li][r0:r0 + 3, 0:32],
                                            gxs[r0:r0 + 3, lb:lb + 512],
                                            start=False, stop=True,
                                            tile_position=(r0, r0))
                        # evacuate raw z to X (fp16) and take stats
                        nc.scalar.activation(X[:, c0:c0 + MACRO], ps[:],
                                             AF.Copy)
                        for b in range(4):
                            nc.vector.bn_stats(stats[:, 4 * t + b, :],
                                               ps[:, 512 * b:512 * b + 512])
                    Cb, s = stats_tail(stats[:], 32, 4, li, m32, bc32, psp, "psA", 128)
                    sprev = bcast_s(s, 32, bc32, psp, "psA", 128, "sprev")
                    # relu pass: X = max(X + C, 0) in place
                    for q in range(4):
                        nc.vector.tensor_scalar(
                            X[:, 8192 * q:8192 * (q + 1)],
                            X[:, 8192 * q:8192 * (q + 1)],
                            Cb[:, 0:1], 0.0, OP.add, OP.max)

            # ================= layer 8 (conv9): stats + maxpool =================
            with tc.tile_pool(name="psL9", bufs=4, space="PSUM") as psp:
                statsL9 = statp.tile([128, 2 * NSLOT, 6], dt.float32, tag="cstats9")
                weff9 = weffp.tile([128, 64], dt.float32, tag="weff9")
                nc.vector.tensor_scalar(weff9[:], wm[8][:], sprev[:, 0:1],
                                        None, OP.mult)
                for t in range(NSLOT):
                    c0 = 512 * t
                    for half in range(2):  # half 0: chunks 0,1 ; half 1: chunks 2,3
                        ps = psp.tile([128, 512], dt.float32, tag="ps9")
                        for jj in range(2):
                            j = 2 * half + jj
                            r0 = 32 * j
                            nc.tensor.matmul(
                                ps[64 * jj:64 * jj + 64, :],
                                weff9[r0:r0 + 32, 0:64],
                                X[r0:r0 + 32, c0:c0 + 512],
                                start=True, stop=True,
                                tile_position=(r0, 64 * jj))
                        nc.vector.bn_stats(statsL9[:, 2 * t + half, :], ps[:])
                        nc.vector.tensor_reduce(
                            pooled[half][:, 16 * t:16 * t + 16],
                            ps[:].rearrange("p (n k) -> p n k", k=K),
                            mybir.AxisListType.X, OP.max)
                Cb9, s9 = stats_tail(statsL9[:], 64, 2, 8, m64, bc64, psp, "ps9", 128)
                s9b = bcast_s(s9, 64, bc64, psp, "ps9", 128, "s9b")
                for half in range(2):
                    nc.vector.tensor_scalar(pooled[half][:], pooled[half][:],
                                            Cb9[:, 0:1], 0.0, OP.add, OP.max)

            # ================= FC stack =================
            with tc.tile_pool(name="psFC", bufs=2, space="PSUM") as psp:
                zfc1 = bigp.tile([64, 2048], dt.float32, tag="zfc1")
                zfc2 = bigp.tile([64, 2048], dt.float32, tag="zfc2")
                xout = bigp.tile([64, 2048], dt.float32, tag="xout")

                wfc1e = weffp.tile([128, 32], dt.float32, tag="wfc1e")
                nc.vector.tensor_scalar(wfc1e[:], wfc1[:], s9b[:, 0:1], None, OP.mult)
                stats1 = statp.tile([64, 4, 6], dt.float32, tag="fstats1")
                for half in range(2):
                    for cs in range(2):
                        ps = psp.tile([128, 512], dt.float32, tag="psfc")
                        nc.tensor.matmul(ps[0:32, :], wfc1e[0:64, 0:32],
                                         pooled[half][0:64, 512 * cs:512 * cs + 512],
                                         start=True, stop=True, tile_position=(0, 0))
                        nc.tensor.matmul(ps[32:64, :], wfc1e[64:128, 0:32],
                                         pooled[half][64:128, 512 * cs:512 * cs + 512],
                                         start=True, stop=True, tile_position=(64, 32))
                        col = 1024 * half + 512 * cs
                        nc.scalar.activation(zfc1[0:64, col:col + 512],
                                             ps[0:64, :], AF.Copy)
                        nc.vector.bn_stats(stats1[0:64, 2 * half + cs, :],
                                           ps[0:64, :])
                Cf1, sf1 = stats_tail(stats1[:], 32, 2, 9, mfc, bcfc, psp, "psfc", 64)
                # Cf1 broadcast is [128,1]; rows 0:64 valid
                for q in range(2):
                    nc.vector.tensor_scalar(zfc1[0:64, 1024 * q:1024 * (q + 1)],
                                            zfc1[0:64, 1024 * q:1024 * (q + 1)],
                                            Cf1[0:64, 0:1], 0.0, OP.add, OP.max)

                sf1b = bcast_s(sf1, 32, bcfc, psp, "psfc", 64, "sf1b")
                wfc2e = weffp.tile([128, 32], dt.float32, tag="wfc2e")
                nc.vector.tensor_scalar(wfc2e[0:64, :], wfc2[0:64, :],
                                        sf1b[0:64, 0:1], None, OP.mult)
                stats2 = statp.tile([64, 4, 6], dt.float32, tag="fstats2")
                for cs in range(4):
                    ps = psp.tile([128, 512], dt.float32, tag="psfc")
                    col = 512 * cs
                    nc.tensor.matmul(ps[0:32, :], wfc2e[0:32, 0:32],
                                     zfc1[0:32, col:col + 512],
                                     start=True, stop=True, tile_position=(0, 0))
                    nc.tensor.matmul(ps[32:64, :], wfc2e[32:64, 0:32],
                                     zfc1[32:64, col:col + 512],
                                     start=True, stop=True, tile_position=(32, 32))
                    nc.scalar.activation(zfc2[0:64, col:col + 512], ps[0:64, :],
                                         AF.Copy)
                    nc.vector.bn_stats(stats2[0:64, cs, :], ps[0:64, :])
                Cf2, sf2 = stats_tail(stats2[:], 32, 2, 10, mfc, bcfc, psp, "psfc", 64)
                for q in range(2):
                    nc.vector.tensor_scalar(zfc2[0:64, 1024 * q:1024 * (q + 1)],
                                            zfc2[0:64, 1024 * q:1024 * (q + 1)],
                                            Cf2[0:64, 0:1], 0.0, OP.add, OP.max)

                sf2b = bcast_s(sf2, 32, bcfc, psp, "psfc", 64, "sf2b")
                wfc3e = weffp.tile([128, 8], dt.float32, tag="wfc3e")
                nc.vector.tensor_scalar(wfc3e[0:64, :], wfc3[0:64, :],
                                        sf2b[0:64, 0:1], None, OP.mult)
                for cs in range(4):
                    ps = psp.tile([128, 512], dt.float32, tag="psfc")
                    col = 512 * cs
                    nc.tensor.matmul(ps[0:8, :], wfc3e[0:32, 0:8],
                                     zfc2[0:32, col:col + 512],
                                     start=True, stop=True, tile_position=(0, 0))
                    nc.tensor.matmul(ps[32:40, :], wfc3e[32:64, 0:8],
                                     zfc2[32:64, col:col + 512],
                                     start=True, stop=True, tile_position=(32, 32))
                    nc.scalar.activation(xout[0:40, col:col + 512], ps[0:40, :],
                                         AF.Identity, bias=bfc3[0:40, 0:1])
                # write out: [6,4096] with global point order
                nc.sync.dma_start(out_d.ap()[:, 0:1024], xout[0:6, 0:1024])
                nc.sync.dma_start(out_d.ap()[:, 1024:2048], xout[32:38, 0:1024])
                nc.sync.dma_start(out_d.ap()[:, 2048:3072], xout[0:6, 1024:2048])
                nc.sync.dma_start(out_d.ap()[:, 3072:4096], xout[32:38, 1024:2048])

    nc.compile()
    return nc


def _pack_host(xyz, neighbors, conv_ws, fc_ws, fc_bs, bn_gs, bn_bs, fcbn_gs, fcbn_bs):
    f16 = np.float32
    f32 = np.float32
    ins = {}
    # masks
    m32 = np.zeros((128, 32), f32)
    for j in range(4):
        m32[32 * j:32 * j + 32] = np.eye(32, dtype=f32)
    m64 = np.zeros((128, 64), f32)
    for g in range(2):
        m64[64 * g:64 * g + 64] = np.eye(64, dtype=f32)
    mfc = np.zeros((64, 32), f32)
    for g in range(2):
        mfc[32 * g:32 * g + 32] = np.eye(32, dtype=f32)
    ins["m32"], ins["m64"], ins["mfc"] = m32, m64, mfc
    ins["bc32"] = m32.T.copy()
    ins["bc64"] = m64.T.copy()
    ins["bcfc"] = mfc.T.copy()
    # conv weights
    for i, (c, o, hasx) in enumerate(CONV):
        W = np.asarray(conv_ws[i], f32)  # [o, c_tot]
        off = 3 if hasx else 0
        wmain = W[:, off:].T  # [c_main, o]
        wt = np.zeros((128, 64 if i == 8 else 32), f16)
        for j in range(4):
            wt[32 * j:32 * j + wmain.shape[0], :o] = wmain.astype(f16)
        ins[f"wm{i}"] = wt
        if hasx:
            wxt = np.zeros((128, 32), f16)
            for j in range(4):
                wxt[32 * j:32 * j + 3, :o] = W[:, 0:3].T.astype(f16)
            ins[f"wx{i}"] = wxt
    # fc weights
    w1 = np.asarray(fc_ws[0], f32).T  # [64, 32]
    wt = np.zeros((128, 32), f16)
    wt[0:64] = w1.astype(f16)
    wt[64:128] = w1.astype(f16)
    ins["wfc1"] = wt
    w2 = np.asarray(fc_ws[1], f32).T  # [32, 32]
    wt = np.zeros((128, 32), f16)
    for g in range(4):
        wt[32 * g:32 * g + 32] = w2.astype(f16)
    ins["wfc2"] = wt
    w3 = np.asarray(fc_ws[2], f32).T  # [32, 6]
    wt = np.zeros((128, 8), f16)
    for g in range(4):
        wt[32 * g:32 * g + 32, 0:6] = w3.astype(f16)
    ins["wfc3"] = wt
    # bn constants: 9 conv + 2 fc
    gam = np.ones((64, 11), f32)
    big = np.zeros((64, 11), f32)
    for l in range(9):
        o = CONV[l][1]
        g = np.asarray(bn_gs[l], f32)
        b = np.asarray(bn_bs[l], f32)
        gam[:o, l] = g
        big[:o, l] = b / g
    for l in range(2):
        g = np.asarray(fcbn_gs[l], f32)
        b = np.asarray(fcbn_bs[l], f32)
        gam[:32, 9 + l] = g
        big[:32, 9 + l] = b / g
    ins["gam"], ins["big"] = gam, big
    bfc3 = np.zeros((64, 1), f32)
    bfc3[0:6, 0] = np.asarray(fc_bs[2], f32)
    bfc3[32:38, 0] = np.asarray(fc_bs[2], f32)
    ins["bfc3"] = bfc3
    return ins


def kernel(xyz, neighbors, conv_ws, conv_bs, bn_gs, bn_bs, fc_ws, fc_bs,
           fcbn_gs, fcbn_bs):
    sys.path.insert(0, "/opt/trn_rl_repo")
    from concourse import bass_utils

    xyz = np.asarray(xyz, np.float32)
    nbr = np.asarray(neighbors).astype(np.int64)

    if "nc" not in _CACHE:
        _CACHE["nc"] = _build()
    nc = _CACHE["nc"]

    shared = _pack_host(xyz, nbr, conv_ws, fc_ws, fc_bs, bn_gs, bn_bs,
                        fcbn_gs, fcbn_bs)
    in_maps = []
    for b in range(B):
        g = xyz[b][nbr[b]]                      # [N, K, 3]
        gt = g.reshape(PAIRS, 3).T              # [3, PAIRS]
        gx = np.ascontiguousarray(
            gt.reshape(3, 4, CHUNK).transpose(1, 0, 2)).astype(np.float32)
        m = dict(shared)
        m["gx"] = gx
        in_maps.append(m)

    res = bass_utils.run_bass_kernel_spmd(nc, in_maps,
                                          core_ids=list(range(NCORES)))
    _CACHE["last_results"] = res

    # host: final Gram-Schmidt axis construction from fc3 output [6, N]
    xs = np.stack([res.results[b]["xout"] for b in range(B)])  # [B, 6, N]
    x = np.ascontiguousarray(xs.transpose(0, 2, 1)).astype(np.float64)  # [B,N,6]
    a1, a2 = x[..., 0:3], x[..., 3:6]
    a1n = np.linalg.norm(a1, axis=-1) + 1e-9
    k = np.sum(a1 * a2, axis=-1) / (a1n ** 2)
    beta2 = a2 - k[..., None] * a1
    x_axis = beta2 / (np.linalg.norm(beta2, axis=-1, keepdims=True) + 1e-9)
    z_axis = a1 / a1n[..., None]
    y_axis = np.cross(z_axis, x_axis)
    return (x_axis.astype(np.float32), y_axis.astype(np.float32),
            z_axis.astype(np.float32))


# revision 10
# speedup vs baseline: 1.1248x; 1.1248x over previous
"""AxisNet Trainium2 kernel.

Sharding: batch-parallel, one batch element per NeuronCore (B=8 = n_cores).
Per core: N=4096 points x K=32 neighbors = 131072 (point,neighbor) pairs,
split into 4 chunks of 32768 pairs living on partition groups [32j:32j+32].

Conv stack (9 shared-MLP layers, training-mode BN): channels-on-partitions,
4-way-concurrent diagonal tile_position matmuls, fp16 activations, BN affine
folded so only relu(z + C) with per-channel C touches the data path (the BN
scale s is folded into the next layer's weights on device).  BN stats are
computed exactly with DVE bn_stats/bn_aggr on the fp32 PSUM conv outputs and
all-reduced across the 8 cores.  Max-pool over K is done on raw conv-9 output
(valid because s>0), then 3 FC layers + BN, final axis Gram-Schmidt on host.
"""

import os
import sys
import numpy as np

B, N, K = 8, 4096, 32
NCORES = 8
PAIRS = N * K            # 131072 per core
CHUNK = PAIRS // 4       # 32768
PTS_PER_CHUNK = N // 4   # 1024
BN_EPS = 1e-5
MACRO = 2048             # pass-A macro tile (4 psum banks)
NMACRO = CHUNK // MACRO  # 16
NSLOT = CHUNK // 512     # 64 bn_stats slots per conv layer

# (c_main, o, has_xyz); L0 reads gx directly
CONV = [
    (3, 6, False), (6, 16, False), (16, 16, False),
    (16, 16, True), (16, 16, False), (16, 32, False),
    (32, 32, True), (32, 32, False), (32, 64, False),
]

_CACHE = {}


def _build(sim_mode=False):
    sys.path.insert(0, "/opt/trn_rl_repo")
    import concourse.bacc as bacc
    import concourse.mybir as mybir
    import concourse.tile as tile

    dt = mybir.dt
    AF = mybir.ActivationFunctionType
    OP = mybir.AluOpType

    nc = bacc.Bacc("TRN2", target_bir_lowering=False, debug=False,
                   enable_asserts=False,
                   num_devices=1 if sim_mode else NCORES)

    # ---------------- DRAM I/O ----------------
    gx_d = nc.dram_tensor("gx", [4, 3, CHUNK], dt.float32, kind="ExternalInput")
    wm_d = [nc.dram_tensor(f"wm{i}", [128, 64 if i == 8 else 32], dt.float32,
                           kind="ExternalInput") for i in range(9)]
    wx_d = {i: nc.dram_tensor(f"wx{i}", [128, 32], dt.float32,
                              kind="ExternalInput") for i in (3, 6)}
    wfc1_d = nc.dram_tensor("wfc1", [128, 32], dt.float32, kind="ExternalInput")
    wfc2_d = nc.dram_tensor("wfc2", [128, 32], dt.float32, kind="ExternalInput")
    wfc3_d = nc.dram_tensor("wfc3", [128, 8], dt.float32, kind="ExternalInput")
    m32_d = nc.dram_tensor("m32", [128, 32], dt.float32, kind="ExternalInput")
    m64_d = nc.dram_tensor("m64", [128, 64], dt.float32, kind="ExternalInput")
    mfc_d = nc.dram_tensor("mfc", [64, 32], dt.float32, kind="ExternalInput")
    bc32_d = nc.dram_tensor("bc32", [32, 128], dt.float32, kind="ExternalInput")
    bc64_d = nc.dram_tensor("bc64", [64, 128], dt.float32, kind="ExternalInput")
    bcfc_d = nc.dram_tensor("bcfc", [32, 64], dt.float32, kind="ExternalInput")
    gam_d = nc.dram_tensor("gam", [64, 11], dt.float32, kind="ExternalInput")
    big_d = nc.dram_tensor("big", [64, 11], dt.float32, kind="ExternalInput")
    bfc3_d = nc.dram_tensor("bfc3", [64, 1], dt.float32, kind="ExternalInput")
    out_d = nc.dram_tensor("xout", [6, 4096], dt.float32, kind="ExternalOutput")

    with tile.TileContext(nc) as tc:
        with tc.tile_pool(name="big", bufs=1) as bigp, \
             tc.tile_pool(name="consts", bufs=1) as cons, \
             tc.tile_pool(name="weff", bufs=2) as weffp, \
             tc.tile_pool(name="gxp", bufs=2) as gxp, \
             tc.tile_pool(name="stat", bufs=1) as statp, \
             tc.tile_pool(name="tiny", bufs=2) as tiny, \
             tc.tile_pool(name="dram", bufs=2, space="DRAM") as drp:

            # --------- resident tensors ---------
            X = bigp.tile([128, CHUNK], dt.float32, tag="X")

            wm = []
            for i in range(9):
                w = cons.tile([128, 64 if i == 8 else 32], dt.float32, tag=f"wm{i}", name=f"wm{i}")
                nc.sync.dma_start(w[:], wm_d[i].ap())
                wm.append(w)
            wx = {}
            for i in (3, 6):
                wx[i] = cons.tile([128, 32], dt.float32, tag=f"wx{i}", name=f"wx{i}")
                nc.sync.dma_start(wx[i][:], wx_d[i].ap())
            wfc1 = cons.tile([128, 32], dt.float32, tag="wfc1")
            wfc2 = cons.tile([128, 32], dt.float32, tag="wfc2")
            wfc3 = cons.tile([128, 8], dt.float32, tag="wfc3")
            nc.sync.dma_start(wfc1[:], wfc1_d.ap())
            nc.sync.dma_start(wfc2[:], wfc2_d.ap())
            nc.sync.dma_start(wfc3[:], wfc3_d.ap())
            m32 = cons.tile([128, 32], dt.float32, tag="m32")
            m64 = cons.tile([128, 64], dt.float32, tag="m64")
            mfc = cons.tile([64, 32], dt.float32, tag="mfc")
            bc32 = cons.tile([32, 128], dt.float32, tag="bc32")
            bc64 = cons.tile([64, 128], dt.float32, tag="bc64")
            bcfc = cons.tile([32, 64], dt.float32, tag="bcfc")
            gam = cons.tile([64, 11], dt.float32, tag="gam")
            big = cons.tile([64, 11], dt.float32, tag="big")
            bfc3 = cons.tile([64, 1], dt.float32, tag="bfc3")
            for t, d in ((m32, m32_d), (m64, m64_d), (mfc, mfc_d),
                         (bc32, bc32_d), (bc64, bc64_d), (bcfc, bcfc_d),
                         (gam, gam_d), (big, big_d), (bfc3, bfc3_d)):
                nc.sync.dma_start(t[:], d.ap())

            pooled = [bigp.tile([128, 1024], dt.float32, tag=f"pool{s}", name=f"pool{s}")
                      for s in range(2)]


            # ---------- stats tail: returns (C_bcast[PB,1]f32, s[m,1]f32) ----------
            def stats_tail(stats_ap, m, G, bn_idx, mask, bcmask, psp, pstag, PB):
                aggr = tiny.tile([128, 2], dt.float32, tag="aggr")
                nc.vector.bn_aggr(aggr[0:stats_ap.shape[0], :], stats_ap)
                # m2 = var + mean^2 (per partition)
                a2 = tiny.tile([128, 2], dt.float32, tag="a2")
                P = stats_ap.shape[0]
                nc.vector.tensor_tensor(a2[0:P, 0:1], aggr[0:P, 0:1],
                                        aggr[0:P, 0:1], OP.mult)
                nc.vector.tensor_tensor(a2[0:P, 1:2], a2[0:P, 0:1],
                                        aggr[0:P, 1:2], OP.add)
                nc.vector.tensor_copy(a2[0:P, 0:1], aggr[0:P, 0:1])
                # sum across groups via mask matmul (fp32)
                psr = psp.tile(psp_shape(pstag), dt.float32, tag=pstag, name="psr")
                nc.tensor.matmul(psr[0:m, 0:2], mask[0:P, 0:m], a2[0:P, 0:2],
                                 start=True, stop=True)
                red = tiny.tile([128, 2], dt.float32, tag="red")
                nc.scalar.activation(red[0:m, 0:2], psr[0:m, 0:2], AF.Copy)
                # cross-core all-reduce
                bin_ = drp.tile([64, 2], dt.float32, tag="ccin")
                bout = drp.tile([64, 2], dt.float32, tag="ccout")
                nc.sync.dma_start(bin_[0:m, :], red[0:m, 0:2])
                if sim_mode:
                    nc.sync.dma_start(bout[:], bin_[:])
                else:
                    nc.gpsimd.collective_compute(
                        "AllReduce", OP.add,
                        replica_groups=[list(range(NCORES))],
                        ins=[bin_.opt()], outs=[bout.opt()])
                rsum = tiny.tile([64, 2], dt.float32, tag="rsum")
                nc.sync.dma_start(rsum[0:m, :], bout[0:m, :])
                inv = 1.0 / (G * NCORES)
                mu = tiny.tile([64, 1], dt.float32, tag="mu")
                m2g = tiny.tile([64, 1], dt.float32, tag="m2g")
                nc.vector.tensor_scalar(mu[0:m, :], rsum[0:m, 0:1], inv, None, OP.mult)
                nc.vector.tensor_scalar(m2g[0:m, :], rsum[0:m, 1:2], inv, None, OP.mult)
                var = tiny.tile([64, 1], dt.float32, tag="var")
                nc.vector.tensor_tensor(var[0:m, :], mu[0:m, :], mu[0:m, :], OP.mult)
                nc.vector.tensor_tensor(var[0:m, :], m2g[0:m, :], var[0:m, :],
                                        OP.subtract)
                nc.vector.tensor_scalar(var[0:m, :], var[0:m, :], BN_EPS, None, OP.add)
                # r = sqrt(var+eps) with one Newton step
                r = tiny.tile([64, 1], dt.float32, tag="r")
                nc.scalar.activation(r[0:m, :], var[0:m, :], AF.Sqrt)
                ir = tiny.tile([64, 1], dt.float32, tag="ir")
                nc.vector.reciprocal(ir[0:m, :], r[0:m, :])
                t1 = tiny.tile([64, 1], dt.float32, tag="t1")
                nc.vector.tensor_tensor(t1[0:m, :], var[0:m, :], ir[0:m, :], OP.mult)
                nc.vector.tensor_tensor(r[0:m, :], r[0:m, :], t1[0:m, :], OP.add)
                nc.vector.tensor_scalar(r[0:m, :], r[0:m, :], 0.5, None, OP.mult)
                # s = gamma / r ; C = (beta/gamma)*r - mu
                s = tiny.tile([64, 1], dt.float32, tag="s")
                nc.vector.reciprocal(ir[0:m, :], r[0:m, :])
                nc.vector.tensor_tensor(s[0:m, :], gam[0:m, bn_idx:bn_idx + 1],
                                        ir[0:m, :], OP.mult)
                Cv = tiny.tile([64, 1], dt.float32, tag="Cv")
                nc.vector.tensor_tensor(Cv[0:m, :], big[0:m, bn_idx:bn_idx + 1],
                                        r[0:m, :], OP.mult)
                nc.vector.tensor_tensor(Cv[0:m, :], Cv[0:m, :], mu[0:m, :],
                                        OP.subtract)
                # broadcast C to [PB,1] via mask matmul
                psb = psp.tile(psp_shape(pstag), dt.float32, tag=pstag, name="psb")
                nc.tensor.matmul(psb[0:PB, 0:1], bcmask[0:m, 0:PB], Cv[0:m, 0:1],
                                 start=True, stop=True)
                Cb = tiny.tile([128, 1], dt.float32, tag="Cb")
                nc.scalar.activation(Cb[0:PB, :], psb[0:PB, 0:1], AF.Copy)
                return Cb, s

            def bcast_s(s, m, bcmask, psp, pstag, PB, tag):
                psb = psp.tile(psp_shape(pstag), dt.float32, tag=pstag, name="psb")
                nc.tensor.matmul(psb[0:PB, 0:1], bcmask[0:m, 0:PB], s[0:m, 0:1],
                                 start=True, stop=True)
                sb = tiny.tile([128, 1], dt.float32, tag=tag, name=tag)
                nc.scalar.activation(sb[0:PB, :], psb[0:PB, 0:1], AF.Copy)
                return sb

            psp_shape = lambda tag: {"psA": [128, MACRO], "ps9": [128, 512],
                                     "psfc": [128, 512]}[tag]

            # ================= conv layers 0..7 =================
            with tc.tile_pool(name="psA", bufs=2, space="PSUM") as psp:
                sprev = None  # s broadcast [128,1] of previous layer
                for li in range(8):
                    c, o, hasx = CONV[li]
                    stats = statp.tile([128, NSLOT, 6], dt.float32, tag="cstats")
                    if li == 0:
                        weff = wm[0]
                    else:
                        weff = weffp.tile([128, 32], dt.float32, tag="weff")
                        nc.vector.tensor_scalar(weff[:], wm[li][:], sprev[:, 0:1],
                                                None, OP.mult)
                    needs_gx = (li == 0) or hasx
                    for t in range(NMACRO):
                        ps = psp.tile([128, MACRO], dt.float32, tag="psA")
                        c0 = t * MACRO
                        if needs_gx:
                            gxs = gxp.tile([128, MACRO], dt.float32, tag="gxs",
                                           name="gxs")
                            for j in range(4):
                                nc.sync.dma_start(
                                    gxs[32 * j:32 * j + 3, :],
                                    gx_d.ap()[j, :, c0:c0 + MACRO])
                        for b in range(4):
                            cb = c0 + 512 * b
                            lb = 512 * b
                            for j in range(4):
                                r0 = 32 * j
                                if li == 0:
                                    nc.tensor.matmul(
                                        ps[r0:r0 + 32, lb:lb + 512],
                                        weff[r0:r0 + c, 0:32],
                                        gxs[r0:r0 + c, lb:lb + 512],
                                        start=True, stop=True,
                                        tile_position=(r0, r0))
                                else:
                                    nc.tensor.matmul(
                                        ps[r0:r0 + 32, lb:lb + 512],
                                        weff[r0:r0 + c, 0:32],
                                        X[r0:r0 + c, cb:cb + 512],
                                        start=True, stop=not hasx,
                                        tile_position=(r0, r0))
                                    if hasx:
                                        nc.tensor.matmul(
                                            ps[r0:r0 + 32, lb:lb + 512],
                                            wx[li][r0:r0 + 3, 0:32],
                                            gxs[r0:r0 + 3, lb:lb + 512],
                                            start=False, stop=True,
                                            tile_position=(r0, r0))
                        # evacuate raw z to X (fp16) and take stats
                        nc.scalar.activation(X[:, c0:c0 + MACRO], ps[:],
                                             AF.Copy)
                        for b in range(4):
                            nc.vector.bn_stats(stats[:, 4 * t + b, :],
                                               ps[:, 512 * b:512 * b + 512])
                    Cb, s = stats_tail(stats[:], 32, 4, li, m32, bc32, psp, "psA", 128)
                    sprev = bcast_s(s, 32, bc32, psp, "psA", 128, "sprev")
                    # relu pass: X = max(X + C, 0) in place (GPSIMD: DVE is stats-bound)
                    for q in range(8):
                        nc.gpsimd.tensor_scalar(
                            X[:, 4096 * q:4096 * (q + 1)],
                            X[:, 4096 * q:4096 * (q + 1)],
                            Cb[:, 0:1], 0.0, OP.add, OP.max)

            # ================= layer 8 (conv9): stats + maxpool =================
            with tc.tile_pool(name="psL9", bufs=4, space="PSUM") as psp:
                statsL9 = statp.tile([128, 2 * NSLOT, 6], dt.float32, tag="cstats9")
                weff9 = weffp.tile([128, 64], dt.float32, tag="weff9")
                nc.vector.tensor_scalar(weff9[:], wm[8][:], sprev[:, 0:1],
                                        None, OP.mult)
                for t in range(NSLOT):
                    c0 = 512 * t
                    for half in range(2):  # half 0: chunks 0,1 ; half 1: chunks 2,3
                        ps = psp.tile([128, 512], dt.float32, tag="ps9")
                        for jj in range(2):
                            j = 2 * half + jj
                            r0 = 32 * j
                            nc.tensor.matmul(
                                ps[64 * jj:64 * jj + 64, :],
                                weff9[r0:r0 + 32, 0:64],
                                X[r0:r0 + 32, c0:c0 + 512],
                                start=True, stop=True,
                                tile_position=(r0, 64 * jj))
                        nc.vector.bn_stats(statsL9[:, 2 * t + half, :], ps[:])
                        nc.vector.tensor_reduce(
                            pooled[half][:, 16 * t:16 * t + 16],
                            ps[:].rearrange("p (n k) -> p n k", k=K),
                            mybir.AxisListType.X, OP.max)
                Cb9, s9 = stats_tail(statsL9[:], 64, 2, 8, m64, bc64, psp, "ps9", 128)
                s9b = bcast_s(s9, 64, bc64, psp, "ps9", 128, "s9b")
                for half in range(2):
                    nc.vector.tensor_scalar(pooled[half][:], pooled[half][:],
                                            Cb9[:, 0:1], 0.0, OP.add, OP.max)

            # ================= FC stack =================
            with tc.tile_pool(name="psFC", bufs=2, space="PSUM") as psp:
                zfc1 = bigp.tile([64, 2048], dt.float32, tag="zfc1")
                zfc2 = bigp.tile([64, 2048], dt.float32, tag="zfc2")
                xout = bigp.tile([64, 2048], dt.float32, tag="xout")

                wfc1e = weffp.tile([128, 32], dt.float32, tag="wfc1e")
                nc.vector.tensor_scalar(wfc1e[:], wfc1[:], s9b[:, 0:1], None, OP.mult)
                stats1 = statp.tile([64, 4, 6], dt.float32, tag="fstats1")
                for half in range(2):
                    for cs in range(2):
                        ps = psp.tile([128, 512], dt.float32, tag="psfc")
                        nc.tensor.matmul(ps[0:32, :], wfc1e[0:64, 0:32],
                                         pooled[half][0:64, 512 * cs:512 * cs + 512],
                                         start=True, stop=True, tile_position=(0, 0))
                        nc.tensor.matmul(ps[32:64, :], wfc1e[64:128, 0:32],
                                         pooled[half][64:128, 512 * cs:512 * cs + 512],
                                         start=True, stop=True, tile_position=(64, 32))
                        col = 1024 * half + 512 * cs
                        nc.scalar.activation(zfc1[0:64, col:col + 512],
                                             ps[0:64, :], AF.Copy)
                        nc.vector.bn_stats(stats1[0:64, 2 * half + cs, :],
                                           ps[0:64, :])
                Cf1, sf1 = stats_tail(stats1[:], 32, 2, 9, mfc, bcfc, psp, "psfc", 64)
                # Cf1 broadcast is [128,1]; rows 0:64 valid
                for q in range(2):
                    nc.vector.tensor_scalar(zfc1[0:64, 1024 * q:1024 * (q + 1)],
                                            zfc1[0:64, 1024 * q:1024 * (q + 1)],
                                            Cf1[0:64, 0:1], 0.0, OP.add, OP.max)

                sf1b = bcast_s(sf1, 32, bcfc, psp, "psfc", 64, "sf1b")
                wfc2e = weffp.tile([128, 32], dt.float32, tag="wfc2e")
                nc.vector.tensor_scalar(wfc2e[0:64, :], wfc2[0:64, :],
                                        sf1b[0:64, 0:1], None, OP.mult)
                stats2 = statp.tile([64, 4, 6], dt.float32, tag="fstats2")
                for cs in range(4):
                    ps = psp.tile([128, 512], dt.float32, tag="psfc")
                    col = 512 * cs
                    nc.tensor.matmul(ps[0:32, :], wfc2e[0:32, 0:32],
                                     zfc1[0:32, col:col + 512],
                                     start=True, stop=True, tile_position=(0, 0))
                    nc.tensor.matmul(ps[32:64, :], wfc2e[32:64, 0:32],
                                     zfc1[32:64, col:col + 512],
                                     start=True, stop=True, tile_position=(32, 32))
                    nc.scalar.activation(zfc2[0:64, col:col + 512], ps[0:64, :],
                                         AF.Copy)
                    nc.vector.bn_stats(stats2[0:64, cs, :], ps[0:64, :])
                Cf2, sf2 = stats_tail(stats2[:], 32, 2, 10, mfc, bcfc, psp, "psfc", 64)
                for q in range(2):
                    nc.vector.tensor_scalar(zfc2[0:64, 1024 * q:1024 * (q + 1)],
                                            zfc2[0:64, 1024 * q:1024 * (q + 1)],
                                            Cf2[0:64, 0:1], 0.0, OP.add, OP.max)

                sf2b = bcast_s(sf2, 32, bcfc, psp, "psfc", 64, "sf2b")
                wfc3e = weffp.tile([128, 8], dt.float32, tag="wfc3e")
                nc.vector.tensor_scalar(wfc3e[0:64, :], wfc3[0:64, :],
                                        sf2b[0:64, 0:1], None, OP.mult)
                for cs in range(4):
                    ps = psp.tile([128, 512], dt.float32, tag="psfc")
                    col = 512 * cs
                    nc.tensor.matmul(ps[0:8, :], wfc3e[0:32, 0:8],
                                     zfc2[0:32, col:col + 512],
                                     start=True, stop=True, tile_position=(0, 0))
                    nc.tensor.matmul(ps[32:40, :], wfc3e[32:64, 0:8],
                                     zfc2[32:64, col:col + 512],
                                     start=True, stop=True, tile_position=(32, 32))
                    nc.scalar.activation(xout[0:40, col:col + 512], ps[0:40, :],
                                         AF.Identity, bias=bfc3[0:40, 0:1])
                # write out: [6,4096] with global point order
                nc.sync.dma_start(out_d.ap()[:, 0:1024], xout[0:6, 0:1024])
                nc.sync.dma_start(out_d.ap()[:, 1024:2048], xout[32:38, 0:1024])
                nc.sync.dma_start(out_d.ap()[:, 2048:3072], xout[0:6, 1024:2048])
                nc.sync.dma_start(out_d.ap()[:, 3072:4096], xout[32:38, 1024:2048])

    nc.compile()
    return nc


def _pack_host(xyz, neighbors, conv_ws, fc_ws, fc_bs, bn_gs, bn_bs, fcbn_gs, fcbn_bs):
    f16 = np.float32
    f32 = np.float32
    ins = {}
    # masks
    m32 = np.zeros((128, 32), f32)
    for j in range(4):
        m32[32 * j:32 * j + 32] = np.eye(32, dtype=f32)
    m64 = np.zeros((128, 64), f32)
    for g in range(2):
        m64[64 * g:64 * g + 64] = np.eye(64, dtype=f32)
    mfc = np.zeros((64, 32), f32)
    for g in range(2):
        mfc[32 * g:32 * g + 32] = np.eye(32, dtype=f32)
    ins["m32"], ins["m64"], ins["mfc"] = m32, m64, mfc
    ins["bc32"] = m32.T.copy()
    ins["bc64"] = m64.T.copy()
    ins["bcfc"] = mfc.T.copy()
    # conv weights
    for i, (c, o, hasx) in enumerate(CONV):
        W = np.asarray(conv_ws[i], f32)  # [o, c_tot]
        off = 3 if hasx else 0
        wmain = W[:, off:].T  # [c_main, o]
        wt = np.zeros((128, 64 if i == 8 else 32), f16)
        for j in range(4):
            wt[32 * j:32 * j + wmain.shape[0], :o] = wmain.astype(f16)
        ins[f"wm{i}"] = wt
        if hasx:
            wxt = np.zeros((128, 32), f16)
            for j in range(4):
                wxt[32 * j:32 * j + 3, :o] = W[:, 0:3].T.astype(f16)
            ins[f"wx{i}"] = wxt
    # fc weights
    w1 = np.asarray(fc_ws[0], f32).T  # [64, 32]
    wt = np.zeros((128, 32), f16)
    wt[0:64] = w1.astype(f16)
    wt[64:128] = w1.astype(f16)
    ins["wfc1"] = wt
    w2 = np.asarray(fc_ws[1], f32).T  # [32, 32]
    wt = np.zeros((128, 32), f16)
    for g in range(4):
        wt[32 * g:32 * g + 32] = w2.astype(f16)
    ins["wfc2"] = wt
    w3 = np.asarray(fc_ws[2], f32).T  # [32, 6]
    wt = np.zeros((128, 8), f16)
    for g in range(4):
        wt[32 * g:32 * g + 32, 0:6] = w3.astype(f16)
    ins["wfc3"] = wt
    # bn constants: 9 conv + 2 fc
    gam = np.ones((64, 11), f32)
    big = np.zeros((64, 11), f32)
    for l in range(9):
        o = CONV[l][1]
        g = np.asarray(bn_gs[l], f32)
        b = np.asarray(bn_bs[l], f32)
        gam[:o, l] = g
        big[:o, l] = b / g
    for l in range(2):
        g = np.asarray(fcbn_gs[l], f32)
        b = np.asarray(fcbn_bs[l], f32)
        gam[:32, 9 + l] = g
        big[:32, 9 + l] = b / g
    ins["gam"], ins["big"] = gam, big
    bfc3 = np.zeros((64, 1), f32)
    bfc3[0:6, 0] = np.asarray(fc_bs[2], f32)
    bfc3[32:38, 0] = np.asarray(fc_bs[2], f32)
    ins["bfc3"] = bfc3
    return ins


def kernel(xyz, neighbors, conv_ws, conv_bs, bn_gs, bn_bs, fc_ws, fc_bs,
           fcbn_gs, fcbn_bs):
    sys.path.insert(0, "/opt/trn_rl_repo")
    from concourse import bass_utils

    xyz = np.asarray(xyz, np.float32)
    nbr = np.asarray(neighbors).astype(np.int64)

    if "nc" not in _CACHE:
        _CACHE["nc"] = _build()
    nc = _CACHE["nc"]

    shared = _pack_host(xyz, nbr, conv_ws, fc_ws, fc_bs, bn_gs, bn_bs,
                        fcbn_gs, fcbn_bs)
    in_maps = []
    for b in range(B):
        g = xyz[b][nbr[b]]                      # [N, K, 3]
        gt = g.reshape(PAIRS, 3).T              # [3, PAIRS]
        gx = np.ascontiguousarray(
            gt.reshape(3, 4, CHUNK).transpose(1, 0, 2)).astype(np.float32)
        m = dict(shared)
        m["gx"] = gx
        in_maps.append(m)

    res = bass_utils.run_bass_kernel_spmd(nc, in_maps,
                                          core_ids=list(range(NCORES)))
    _CACHE["last_results"] = res

    # host: final Gram-Schmidt axis construction from fc3 output [6, N]
    xs = np.stack([res.results[b]["xout"] for b in range(B)])  # [B, 6, N]
    x = np.ascontiguousarray(xs.transpose(0, 2, 1)).astype(np.float64)  # [B,N,6]
    a1, a2 = x[..., 0:3], x[..., 3:6]
    a1n = np.linalg.norm(a1, axis=-1) + 1e-9
    k = np.sum(a1 * a2, axis=-1) / (a1n ** 2)
    beta2 = a2 - k[..., None] * a1
    x_axis = beta2 / (np.linalg.norm(beta2, axis=-1, keepdims=True) + 1e-9)
    z_axis = a1 / a1n[..., None]
    y_axis = np.cross(z_axis, x_axis)
    return (x_axis.astype(np.float32), y_axis.astype(np.float32),
            z_axis.astype(np.float32))
